# revision 9
# baseline (speedup 1.0000x reference)
"""GCNConv (add_self_loops=False, normalize=False) on 8 TRN2 NeuronCores.

out = scatter_add(x[src] -> dst) @ W + b

Strategy (owner-computes, no collectives):
- Nodes dst-sharded: core c owns rows [c*12500, (c+1)*12500).
- Per core, nodes split into 110 tiles (113/114 nodes each). Edges bucketed
  by (dst tile, src quarter); aggregated in x-space via one-hot matmuls into
  PSUM (aggT[d, n]), then a per-tile GEMM aggT.T @ W + b.
- x converted to fp16 on host; gathered by row via gpsimd dma_gather (int16
  indices -> x split into 4 quarter tables of 25000 rows). Gather calls are
  capped at 8 chunks = 1024 indices (HW descriptor-ring limit).
- One-hot built on DVE: is_equal(dstl broadcast, iota broadcast), fp16.
- All cores run ONE program; per-(tile,quarter) chunk counts are the max
  over cores, shortfall padded with (idx=0, dstl=SENTINEL) slots whose
  one-hot rows are all zero.
"""
import os
import sys
sys.path.insert(0, "/opt/trn_rl_repo")

import numpy as np

N_NODES = 100000
N_EDGES = 1600000
DIM = 128
P = 128
N_CORES = 8
NS = N_NODES // N_CORES          # 12500 nodes per core
NTILES = 110                     # node tiles per core (113/114 nodes)
QS = 25000                       # quarter table rows (int16-addressable)
NQ = 4
CALL_CHUNKS = 8                  # max chunks (x128 idxs) per dma_gather call
SENT = 400.0                     # dstl sentinel for pad slots (>=128)

_tb = [(NS * t) // NTILES for t in range(NTILES + 1)]   # tile boundaries


def _build_schedule(src, dst):
    """Bucket edges; return schedule + per-core packed arrays."""
    octant = dst // NS
    r = dst - octant * NS
    tb = np.asarray(_tb, np.int64)
    tile = np.searchsorted(tb, r, side="right") - 1
    dstl = r - tb[tile]
    q = src // QS
    qsrc = (src - q * QS).astype(np.int16)

    key = ((octant * NTILES + tile) * NQ + q).astype(np.int64)
    order = np.argsort(key, kind="stable")
    qsrc_s = qsrc[order]
    dstl_s = dstl[order].astype(np.float16)
    ncell = N_CORES * NTILES * NQ
    counts = np.bincount(key, minlength=ncell).reshape(N_CORES, NTILES, NQ)
    starts = np.zeros(ncell + 1, np.int64)
    np.cumsum(counts.reshape(-1), out=starts[1:])

    chunks = -(-counts.max(axis=0) // P)             # [NTILES, NQ]
    chunks = np.maximum(chunks, (counts.max(axis=0) > 0).astype(chunks.dtype))

    # one-hot/dstl layout: tile-major columns, q-groups in order within a tile
    qcol = np.zeros((NTILES, NQ), np.int64)
    np.cumsum(chunks[:, :-1], axis=1, out=qcol[:, 1:])
    C_t = chunks.sum(axis=1)
    TCO = np.zeros(NTILES + 1, np.int64)
    np.cumsum(C_t, out=TCO[1:])
    C_TOT = int(TCO[-1])

    # per-quarter chunk streams: global position of (t,q)'s first chunk
    P0 = np.zeros((NTILES, NQ), np.int64)
    TQ = np.zeros(NQ, np.int64)
    for qi in range(NQ):
        pos = 0
        for t in range(NTILES):
            P0[t, qi] = pos
            pos += chunks[t, qi]
        TQ[qi] = pos
    QBASE = np.zeros(NQ + 1, np.int64)               # idx col base per q area
    np.cumsum(TQ * CALL_CHUNKS, out=QBASE[1:])       # 8 idx cols per chunk
    IDXCOLS = int(QBASE[-1])

    dstl_all = np.full((N_CORES, P, max(C_TOT, 1)), SENT, np.float16)
    idx16 = np.zeros((N_CORES, 16, max(IDXCOLS, 1)), np.int16)

    for c in range(N_CORES):
        for t in range(NTILES):
            for qi in range(NQ):
                cell = (c * NTILES + t) * NQ + qi
                cnt = int(counts[c, t, qi])
                if cnt == 0:
                    continue
                sl = slice(starts[cell], starts[cell] + cnt)
                i = np.arange(cnt)
                cols = TCO[t] + qcol[t, qi] + i // P
                dstl_all[c, i % P, cols] = dstl_s[sl]
                j = (P0[t, qi] + i // P) * P + i % P
                idx16[c, j % 16, QBASE[qi] + j // 16] = qsrc_s[sl]
    return dict(chunks=chunks, qcol=qcol, C_t=C_t, TCO=TCO, C_TOT=C_TOT,
                P0=P0, TQ=TQ, QBASE=QBASE, IDXCOLS=IDXCOLS,
                dstl_all=dstl_all, idx16=idx16)


def _build_program(S):
    import concourse.bacc as bacc
    import concourse.mybir as mybir
    import concourse.tile as tile

    dt = mybir.dt
    nqueues = 1
    nc = bacc.Bacc("TRN2", target_bir_lowering=False, num_swdge_queues=nqueues)
    xq_d = [nc.dram_tensor(f"xq{q}", [QS, DIM], dt.float16, kind="ExternalInput")
            for q in range(NQ)]
    idx_d = nc.dram_tensor("idx", [P, S["IDXCOLS"]], dt.int16, kind="ExternalInput")
    di_d = nc.dram_tensor("di", [P, S["C_TOT"] + P], dt.float16, kind="ExternalInput")
    wob_d = nc.dram_tensor("wob", [DIM, 3 * DIM], dt.float32, kind="ExternalInput")
    out_d = nc.dram_tensor("out", [NS, DIM], dt.float32, kind="ExternalOutput")

    chunks, qcol, TCO = S["chunks"], S["qcol"], S["TCO"]
    P0, TQ, QBASE = S["P0"], S["TQ"], S["QBASE"]
    ntile_lim = int(os.environ.get("K_NTILES", "0")) or NTILES
    act_copy = os.environ.get("K_ACT_COPY", "1") == "1"
    gbufs = int(os.environ.get("K_GBUFS", "6"))
    hbufs = int(os.environ.get("K_HBUFS", "4"))
    pbufs = int(os.environ.get("K_PBUFS", "4"))

    with tile.TileContext(nc) as tc:
        with tc.tile_pool(name="cpool", bufs=1) as cpool, \
             tc.tile_pool(name="gpool", bufs=gbufs) as gpool, \
             tc.tile_pool(name="hpool", bufs=hbufs) as hpool, \
             tc.tile_pool(name="opool", bufs=hbufs) as opool, \
             tc.tile_pool(name="ppool", bufs=pbufs, space="PSUM") as ppool:
            idx_s = cpool.tile([P, S["IDXCOLS"]], dt.int16)
            nc.sync.dma_start(idx_s[:, :], idx_d[:, :])
            di_s = cpool.tile([P, S["C_TOT"] + P], dt.float16)
            nc.sync.dma_start(di_s[:, :], di_d[:, :])
            wob_s = cpool.tile([DIM, 3 * DIM], dt.float32)
            nc.sync.dma_start(wob_s[:, :], wob_d[:, :])
            w_ap = wob_s[:, :DIM]
            ones_ap = wob_s[0:1, DIM:2 * DIM]
            b_ap = wob_s[0:1, 2 * DIM:3 * DIM]
            dstl_ap = di_s[:, :S["C_TOT"]]
            iota_ap = di_s[:, S["C_TOT"]:]

            emitted = [0] * NQ                    # next call id to emit, per q
            bufs = [dict() for _ in range(NQ)]    # call id -> tile AP

            def ensure_call(qi, call):
                while emitted[qi] <= call:
                    cid = emitted[qi]
                    cols = int(min(CALL_CHUNKS, TQ[qi] - cid * CALL_CHUNKS))
                    g = gpool.tile([P, cols, DIM], dt.float16, tag=f"xg{qi}",
                                   name=f"xg{qi}_{cid}")
                    ic0 = int(QBASE[qi] + cid * CALL_CHUNKS * 8)
                    nc.gpsimd.dma_gather(
                        out_ap=g[:, :, :],
                        in_ap=xq_d[qi][:, :],
                        idxs_ap=idx_s[:, ic0:ic0 + cols * 8],
                        num_idxs=cols * P,
                        num_idxs_reg=cols * P,
                        elem_size=DIM,
                        
                    )
                    bufs[qi][cid] = g
                    if cid - 4 in bufs[qi]:
                        del bufs[qi][cid - 4]
                    emitted[qi] += 1

            for t in range(ntile_lim):
                C_t = int(S["C_t"][t])
                if C_t == 0:
                    continue
                for qi in range(NQ):
                    nch = int(chunks[t, qi])
                    if nch:
                        last_pos = int(P0[t, qi]) + nch - 1
                        ensure_call(qi, last_pos // CALL_CHUNKS)
                oh = hpool.tile([P, C_t, P], dt.float16, tag="oh", name=f"oh_{t}")
                oh_split = int(os.environ.get("K_OH_SPLIT", "1"))
                hseg = max(1, (C_t + oh_split - 1) // oh_split)
                for s0 in range(0, C_t, hseg):
                    s1 = min(s0 + hseg, C_t)
                    nc.vector.tensor_tensor(
                        out=oh[:, s0:s1, :],
                        in0=dstl_ap[:, TCO[t] + s0:TCO[t] + s1, None].to_broadcast([P, s1 - s0, P]),
                        in1=iota_ap[:, None, :].to_broadcast([P, s1 - s0, P]),
                        op=mybir.AluOpType.is_equal,
                    )
                aggT_p = ppool.tile([DIM, P], dt.float32, tag="agg", name=f"agg_{t}")
                n_emitted = 0
                for qi in range(NQ):
                    for cci in range(int(chunks[t, qi])):
                        pos = int(P0[t, qi]) + cci
                        g = bufs[qi][pos // CALL_CHUNKS]
                        n_emitted += 1
                        nc.tensor.matmul(
                            aggT_p[:, :],
                            g[:, pos % CALL_CHUNKS, :],
                            oh[:, int(qcol[t, qi]) + cci, :],
                            start=(n_emitted == 1), stop=(n_emitted == C_t),
                        )
                aggT_s = opool.tile([DIM, P], dt.float32, tag="aggs", name=f"aggs_{t}")
                if act_copy:
                    nc.scalar.activation(aggT_s[:, :], aggT_p[:, :],
                                         func=mybir.ActivationFunctionType.Copy)
                else:
                    nc.vector.tensor_copy(aggT_s[:, :], aggT_p[:, :])
                out_p = ppool.tile([P, DIM], dt.float32, tag="outp", name=f"outp_{t}")
                nc.tensor.matmul(out_p[:, :], aggT_s[:, :], w_ap,
                                 start=True, stop=False)
                nc.tensor.matmul(out_p[:, :], ones_ap, b_ap,
                                 start=False, stop=True)
                out_s = opool.tile([P, DIM], dt.float32, tag="outs", name=f"outs_{t}")
                if act_copy:
                    nc.scalar.activation(out_s[:, :], out_p[:, :],
                                         func=mybir.ActivationFunctionType.Copy)
                else:
                    nc.vector.tensor_copy(out_s[:, :], out_p[:, :])
                rows = _tb[t + 1] - _tb[t]
                nc.sync.dma_start(out_d[_tb[t]:_tb[t + 1], :], out_s[:rows, :])
    nc.finalize()
    return nc


def _run(inputs, trace=False, n_cores=N_CORES):
    from concourse.bass_utils import run_bass_kernel_spmd

    x = np.asarray(inputs["x"], np.float32)
    edge_index = np.asarray(inputs["edge_index"])
    W = np.asarray(inputs["W"], np.float32)
    b = np.asarray(inputs["b"], np.float32)

    src = edge_index[0].astype(np.int64)
    dst = edge_index[1].astype(np.int64)
    S = _build_schedule(src, dst)
    nc = _build_program(S)

    x16 = x.astype(np.float16)
    xqs = [np.ascontiguousarray(x16[q * QS:(q + 1) * QS]) for q in range(NQ)]
    iota = np.tile(np.arange(P, dtype=np.float16), (P, 1))
    wob = np.zeros((DIM, 3 * DIM), np.float32)
    wob[:, :DIM] = W
    wob[0, DIM:2 * DIM] = 1.0
    wob[0, 2 * DIM:3 * DIM] = b

    in_maps = []
    for c in range(N_CORES):
        di = np.concatenate([S["dstl_all"][c], iota], axis=1)
        m = {f"xq{q}": xqs[q] for q in range(NQ)}
        m["idx"] = np.tile(S["idx16"][c], (8, 1))
        m["di"] = np.ascontiguousarray(di)
        m["wob"] = wob
        in_maps.append(m)

    res = run_bass_kernel_spmd(nc, in_maps[:n_cores], core_ids=list(range(n_cores)),
                               trace=trace)
    out = np.concatenate([res.results[c]["out"] for c in range(n_cores)], axis=0)
    return out, res


def kernel(**inputs):
    out, _ = _run(inputs, trace=False)
    return (out, (None, None))
